# revision 20
# baseline (speedup 1.0000x reference)
"""Trainium2 Bass kernel for nn_CrossAttention (b=2, s1=2048, s2=1024, H=16, hd=64).

Sharding: 8 cores = 2 batches x 4 head-groups (4 heads each).

Per-core device program (bf16 matmul datapath, fp32 PSUM accumulation),
restructured from the phase-serialized baseline into one dense PE stream so
the tensor engine stays at max p-state:

  - qproj (k-outer, fat x-row streams on the sync DGE ring)
  - LN stats (selector matmuls) emitted immediately per slice; their DVE/ACT
    chains (var, gathered approx-reciprocal, sqrt) overlap kproj/vproj on PE
  - LN apply fused: w baked into the broadcast selector on host, so
    apply = src*psA + (psB + b) = one tensor_tensor + one scalar_tensor_tensor
    reading the broadcast PSUM directly (no Ap/Bp materialization)
  - RoPE on q in place (head-dim pre-permuted; even/odd swap via SBUF DMAs)
  - attention in two 1024-column windows (np): per (head, m-chunk):
    scoresT -> exp (ACT, scale fused) -> AV accumulate, software-pipelined
    with a 4-deep [128,512] PSUM ring (scores + recip-broadcast + out-proj
    tiles share it) and a 2-deep [65,1024] AV-accumulator ring
  - softmax denominator rides as the ones-column of v_aug (row 64 of the AV
    accumulator); per (head, window): gpsimd row-copy -> gather-DMA ->
    approx-reciprocal -> scatter -> K=1 ones-matmul broadcast; the normalize
    multiply drains the AV PSUM directly to SBUF (no separate copy)
  - out-proj of window 0 interleaved into window 1's score stream; outputs
    stream to HBM as bf16 partials (summed + out_b on host)

The emitted BIR is post-processed to split multi-semaphore waits into
single-wait NOP chains (this walrus build allows only 1 sync-wait on
self-loading matmults and very few elsewhere).
"""

import numpy as np

B, S1, S2, CIN, H, HD = 2, 2048, 1024, 1024, 16, 64
HPC = 4                # heads per core
CPC = HPC * HD         # 256 channels per core
P = 128
KC = CIN // P          # 8 cin chunks
MC = CPC // P          # 2 channel chunks
NQ = S1 // 512         # 4 s1 slices
NK = S2 // 512         # 2 s2 slices
M2 = S2 // P           # 8 s2 chunks
NP = 2                 # attention windows of 1024 columns
SCALE = HD ** -0.5
EPS = 1e-6

_NC_CACHE = {}


def _legalize_waits(nc, mybir, limit=1):
    """Split instructions carrying >limit semaphore waits into a chain of
    single-wait NOPs on the same engine followed by the instruction."""
    n_split = 0
    for fn in nc.m.functions:
        for bb in fn.blocks:
            out = []
            for inst in bb.instructions:
                si = inst.sync_info
                waits = list(si.on_wait) if si is not None and si.on_wait else []
                if len(waits) > limit:
                    for i, w in enumerate(waits[:-limit]):
                        nop = mybir.InstNoOp(
                            name=f"{inst.name}-lw{i}", ins=[], outs=[])
                        nop.engine = inst.engine
                        nop.sync_info = mybir.SyncInfo(on_wait=[w], on_update=[])
                        try:
                            nc.register_instruction(nop, overwrite=True)
                        except Exception:
                            pass
                        out.append(nop)
                    inst.sync_info = mybir.SyncInfo(
                        on_wait=waits[-limit:], on_update=list(si.on_update))
                    n_split += 1
                out.append(inst)
            bb.instructions = out
    return n_split


def _build_nc():
    from contextlib import ExitStack

    import concourse.bass as bass
    import concourse.mybir as mybir
    import concourse.tile as tile

    f32 = mybir.dt.float32
    bf16 = mybir.dt.bfloat16
    AF = mybir.ActivationFunctionType
    OP = mybir.AluOpType

    nc = bass.Bass()

    def din(name, shape, dt=bf16):
        return nc.dram_tensor(name, list(shape), dt, kind="ExternalInput")

    xT = din("xT", (CIN, S1))
    yT = din("yT", (CIN, S2))
    qwt = din("qwt", (CIN, CPC))
    kwt = din("kwt", (CIN, CPC))
    vwt = din("vwt", (CIN, CPC))
    owt = din("owt", (CPC, CIN))
    qb = din("qb", (CPC,), f32)
    kb = din("kb", (CPC,), f32)
    vb = din("vb", (CPC,), f32)
    qnb = din("qnb", (CPC,), f32)
    knb = din("knb", (CPC,), f32)
    cosf = din("cosf", (P, S1))
    sinf = din("sinf", (P, S1))
    seld = din("sel", (CPC, HPC))
    rselq = din("rselq", (HPC, CPC))   # selector rows carry qn_w
    rselk = din("rselk", (HPC, CPC))   # selector rows carry kn_w
    outT = nc.dram_tensor("outT", [CIN, S1], bf16, kind="ExternalOutput")

    with tile.TileContext(nc) as tc, ExitStack() as ctx:
        consts = ctx.enter_context(tc.tile_pool(name="consts", bufs=1))
        pers = ctx.enter_context(tc.tile_pool(name="pers", bufs=1))
        xs = ctx.enter_context(tc.tile_pool(name="xs", bufs=4))
        tmp = ctx.enter_context(tc.tile_pool(name="tmp", bufs=3))
        rop = ctx.enter_context(tc.tile_pool(name="rop", bufs=3))
        expp = ctx.enter_context(tc.tile_pool(name="expp", bufs=3))
        # ---- PSUM pools, projection phase (closed before attention) ----
        ctxA = ctx.enter_context(ExitStack())
        acc = ctxA.enter_context(tc.tile_pool(name="acc", bufs=4, space="PSUM"))
        pst = ctxA.enter_context(tc.tile_pool(name="pst", bufs=2, space="PSUM"))
        pvv = ctxA.enter_context(tc.tile_pool(name="pvv", bufs=2, space="PSUM"))

        # ---- constants ----
        qwt_sb = consts.tile([P, KC, CPC], bf16)
        nc.sync.dma_start(out=qwt_sb, in_=qwt.rearrange("(k p) m -> p k m", p=P))
        sel_sb = consts.tile([P, MC, HPC], bf16)
        nc.scalar.dma_start(out=sel_sb, in_=seld.rearrange("(c p) h -> p c h", p=P))
        rselq_sb = consts.tile([HPC, CPC], bf16)
        nc.scalar.dma_start(out=rselq_sb, in_=rselq[:])
        rselk_sb = consts.tile([HPC, CPC], bf16)
        nc.scalar.dma_start(out=rselk_sb, in_=rselk[:])
        kwt_sb = consts.tile([P, KC, CPC], bf16)
        nc.scalar.dma_start(out=kwt_sb, in_=kwt.rearrange("(k p) m -> p k m", p=P))
        yT_sb = consts.tile([P, KC, S2], bf16)
        nc.scalar.dma_start(out=yT_sb, in_=yT.rearrange("(k p) s -> p k s", p=P))
        vwt_sb = consts.tile([P, KC, CPC], bf16)
        nc.scalar.dma_start(out=vwt_sb, in_=vwt.rearrange("(k p) m -> p k m", p=P))
        cosf_sb = consts.tile([P, S1], bf16)
        nc.scalar.dma_start(out=cosf_sb, in_=cosf[:])
        sinf_sb = consts.tile([P, S1], bf16)
        nc.scalar.dma_start(out=sinf_sb, in_=sinf[:])
        owt_sb = consts.tile([P, MC, CIN], bf16)
        nc.scalar.dma_start(out=owt_sb, in_=owt.rearrange("(c p) m -> p c m", p=P))

        def perpart(name, d, eng=None):
            t = consts.tile([P, MC], f32, name=name)
            (eng or nc.scalar).dma_start(out=t, in_=d.rearrange("(c p) -> p c", p=P))
            return t

        qb_sb = perpart("qb_sb", qb)
        kb_sb = perpart("kb_sb", kb)
        qnb_sb = perpart("qnb_sb", qnb)
        knb_sb = perpart("knb_sb", knb)
        ones64 = consts.tile([1, 64], bf16)
        nc.vector.memset(ones64, 1.0)
        # v bias broadcast across all partitions
        vbb_sb = consts.tile([P, CPC], f32)
        vb_ap = vb[:]
        nc.gpsimd.dma_start(
            out=vbb_sb,
            in_=bass.AP(tensor=vb_ap.tensor, offset=vb_ap.offset,
                        ap=[[0, P]] + list(vb_ap.ap)),
        )

        # ---- persistent activations ----
        qT_sb = pers.tile([P, MC, S1], bf16)
        kT_sb = pers.tile([P, MC, S2], bf16)
        v_sb = pers.tile([P, M2, HPC, HD + 1], bf16)
        onorm = pers.tile([P, MC, S1], bf16)
        # squared copies for LN variance
        sq_q = pers.tile([P, MC, S1], bf16)
        sq_k = pers.tile([P, MC, S2], bf16)
        # LN stat staging: [32,64] gathered blocks per slice-group
        lncoll = pers.tile([P, 64], f32)
        lnrc = pers.tile([P, 64], f32)
        lnrcb = pers.tile([P, 64], f32)

        # =============== q projection: qT[c*128+p, s1] ===============
        for half in range(2):
            hsl = slice(half * 1024, (half + 1) * 1024)
            ps = [[acc.tile([P, 512], f32, name=f"psq{c}_{half}{n}", tag="acc")
                   for n in range(2)] for c in range(MC)]
            for k in range(KC):
                xt = xs.tile([P, 1024], bf16, name=f"xt{half}_{k}", tag="xs")
                nc.sync.dma_start(out=xt, in_=xT[k * P:(k + 1) * P, hsl])
                for c in range(MC):
                    for n in range(2):
                        nc.tensor.matmul(
                            ps[c][n][:], qwt_sb[:, k, c * P:(c + 1) * P],
                            xt[:, n * 512:(n + 1) * 512],
                            start=(k == 0), stop=(k == KC - 1))
            for c in range(MC):
                for n in range(2):
                    gn = half * 2 + n
                    sl = slice(gn * 512, (gn + 1) * 512)
                    nc.scalar.activation(out=qT_sb[:, c, sl], in_=ps[c][n][:],
                                         func=AF.Identity,
                                         bias=qb_sb[:, c:c + 1])
                    nc.scalar.activation(out=sq_q[:, c, sl],
                                         in_=qT_sb[:, c, sl], func=AF.Square)

        # =============== LN machinery ===============
        # Stats per 512-slice group g (q: g=0..3, k: g=4..5):
        #   pss[h,pos] = sum over head channels, psq = sum of squares (PE sel)
        #   var on DVE; gathered to [32,64]: +eps, approx-recip, ACT sqrt ->
        #   rstd; scattered back as Ab [4,512] bf16; Bb = -mu*rstd.
        stat_ps = {}

        def ln_stats_mm(src, sq, g, n):
            pss = pst.tile([HPC, 512], f32, name=f"pss{g}", tag="pss", bufs=1)
            psq = pst.tile([HPC, 512], f32, name=f"psq{g}", tag="psq", bufs=1)
            sl = slice(n * 512, (n + 1) * 512)
            for c in range(MC):
                nc.tensor.matmul(pss[:], sel_sb[:, c, :], src[:, c, sl],
                                 start=(c == 0), stop=(c == MC - 1))
            for c in range(MC):
                nc.tensor.matmul(psq[:], sel_sb[:, c, :], sq[:, c, sl],
                                 start=(c == 0), stop=(c == MC - 1))
            stat_ps[g] = (pss, psq)

        def ln_stats_dve(g):
            """Emits DVE/gpsimd/ACT chain; returns (Ab, Bb) [4,512] bf16."""
            pss, psq = stat_ps[g]
            mu = tmp.tile([HPC, 512], f32, name=f"mu{g}", tag="mu", bufs=2)
            var = tmp.tile([HPC, 512], f32, name=f"var{g}", tag="var", bufs=2)
            Af = tmp.tile([HPC, 512], f32, name=f"Af{g}", tag="Af", bufs=2)
            Ab = tmp.tile([HPC, 512], bf16, name=f"Ab{g}", tag="Ab", bufs=2)
            Bb = tmp.tile([HPC, 512], bf16, name=f"Bb{g}", tag="Bb", bufs=2)
            nc.scalar.mul(mu[:], pss[:], 1.0 / HD)
            nc.scalar.activation(out=var[:], in_=mu[:], func=AF.Square)
            # var = psq/HD - mu^2
            nc.vector.scalar_tensor_tensor(
                out=var[:], in0=psq[:], scalar=1.0 / HD, in1=var[:],
                op0=OP.mult, op1=OP.subtract)
            # gather (4,512) -> 32 partitions so recip/sqrt are cheap
            r0 = g * 32 if g < 4 else (g - 4) * 32
            coll = lncoll if g < 4 else lnrcb
            rc = lnrc
            rr = slice(r0, r0 + 32)
            nc.gpsimd.dma_start(out=coll[rr, :], in_=var[:])
            nc.vector.tensor_scalar_add(coll[rr, :], coll[rr, :], float(EPS))
            nc.vector.reciprocal(rc[rr, :], coll[rr, :])
            # rstd = sqrt(1/(var+eps))
            nc.scalar.activation(out=rc[rr, :], in_=rc[rr, :], func=AF.Sqrt)
            nc.gpsimd.dma_start(out=Af[:], in_=rc[rr, :])
            nc.scalar.copy(Ab[:], Af[:])
            nc.vector.scalar_tensor_tensor(
                out=Bb[:], in0=mu[:], scalar=-1.0, in1=Af[:],
                op0=OP.mult, op1=OP.mult)                     # B = -mu*rstd
            return Ab, Bb

        def ln_bcast_mm(rsel_sb, Ab, Bb, g, c):
            """PE broadcast of per-head A,B to all channels (w baked in)."""
            psA = acc.tile([P, 512], f32, name=f"psA{g}_{c}", tag="acc")
            nc.tensor.matmul(psA[:], rsel_sb[:, c * P:(c + 1) * P], Ab[:],
                             start=True, stop=True)
            psB = acc.tile([P, 512], f32, name=f"psB{g}_{c}", tag="acc")
            nc.tensor.matmul(psB[:], rsel_sb[:, c * P:(c + 1) * P], Bb[:],
                             start=True, stop=True)
            return psA, psB

        def ln_apply_dve(src, b_sb, psA, psB, n, c):
            sl = slice(n * 512, (n + 1) * 512)
            nc.vector.tensor_mul(src[:, c, sl], src[:, c, sl], psA[:])
            # src = (psB + b) + src
            nc.vector.scalar_tensor_tensor(
                out=src[:, c, sl], in0=psB[:], scalar=b_sb[:, c:c + 1],
                in1=src[:, c, sl], op0=OP.add, op1=OP.add)

        def kproj_n(n):
            sl = slice(n * 512, (n + 1) * 512)
            ps = [acc.tile([P, 512], f32, name=f"psk{c}_{n}", tag="acc")
                  for c in range(MC)]
            for k in range(KC):
                for c in range(MC):
                    nc.tensor.matmul(
                        ps[c][:], kwt_sb[:, k, c * P:(c + 1) * P],
                        yT_sb[:, k, sl],
                        start=(k == 0), stop=(k == KC - 1))
            for c in range(MC):
                nc.scalar.activation(out=kT_sb[:, c, sl], in_=ps[c][:],
                                     func=AF.Identity, bias=kb_sb[:, c:c + 1])
                nc.scalar.activation(out=sq_k[:, c, sl], in_=kT_sb[:, c, sl],
                                     func=AF.Square)

        # Per-slice LN: stats -> cross-engine chain -> broadcast+apply. The
        # broadcast matmuls wait on the chain, giving the PE a ~2-3us idle
        # window per slice -- intentional: these gaps drain the hardware
        # activity monitor so the dense sprints run at full clock.
        def ln_slice(src, sq, b_sb, rsel_sb, g, n):
            ln_stats_mm(src, sq, g, n)
            Ab, Bb = ln_stats_dve(g)
            for c in range(MC):
                psA, psB = ln_bcast_mm(rsel_sb, Ab, Bb, g, c)
                ln_apply_dve(src, b_sb, psA, psB, n, c)

        for g in range(2):
            ln_slice(qT_sb, sq_q, qnb_sb, rselq_sb, g, g)
        kproj_n(0)
        ln_slice(qT_sb, sq_q, qnb_sb, rselq_sb, 2, 2)
        kproj_n(1)
        ln_slice(qT_sb, sq_q, qnb_sb, rselq_sb, 3, 3)
        ln_slice(kT_sb, sq_k, knb_sb, rselk_sb, 4, 0)
        ln_slice(kT_sb, sq_k, knb_sb, rselk_sb, 5, 1)

        qsws = {}

        def rope_swap(c):
            qsw = rop.tile([P, S1], bf16, name=f"qsw{c}", tag=f"qsw{c}",
                           bufs=1)
            for blk in range(4):
                d_src = (blk ^ 1) * 32
                nc.scalar.dma_start(out=qsw[blk * 32:(blk + 1) * 32, :],
                                    in_=qT_sb[d_src:d_src + 32, c, :])
            qsws[c] = qsw

        def rope_slice(c, n):
            qsw = qsws[c]
            sl = slice(n * 512, (n + 1) * 512)
            t = rop.tile([P, 512], bf16, name=f"rt{c}_{n}", tag="rt")
            nc.vector.tensor_mul(t[:], qsw[:, sl], sinf_sb[:, sl])
            nc.vector.tensor_mul(qT_sb[:, c, sl], qT_sb[:, c, sl],
                                 cosf_sb[:, sl])
            nc.vector.tensor_add(qT_sb[:, c, sl], qT_sb[:, c, sl], t[:])

        # ======== v projection sprint (RoPE on DVE runs in its shadow) ======
        # Partition layout per q chunk: [h_a evens | h_a odds | h_b evens |
        # h_b odds] (32 each). even<->odd swap via SBUF DMAs; sign baked
        # into the sin table.
        nc.vector.memset(v_sb[:, :, :, HD:HD + 1], 1.0)
        rope_swap(0)
        for m in range(M2):
            psv = pvv.tile([P, CPC], f32, name=f"psv{m}", tag="pvv")
            for k in range(KC):
                nc.tensor.matmul(
                    psv[:], yT_sb[:, k, m * P:(m + 1) * P], vwt_sb[:, k, :],
                    start=(k == 0), stop=(k == KC - 1))
            nc.vector.tensor_add(
                v_sb[:, m, :, 0:HD],
                psv.rearrange("p (h d) -> p h d", h=HPC),
                vbb_sb.rearrange("p (h d) -> p h d", h=HPC))
            if m == 3:
                rope_swap(1)
            if m < NQ:
                rope_slice(0, m)
            else:
                rope_slice(1, m - NQ)

        # =============== attention + out-projection ===============
        ctxA.close()   # free proj/LN psum banks
        ctxB = ctx.enter_context(ExitStack())
        # ring: scores (later recip-broadcast / out-proj) tiles, 4 x [128,512]
        ring = ctxB.enter_context(tc.tile_pool(name="ring", bufs=4, space="PSUM"))
        psoP = ctxB.enter_context(tc.tile_pool(name="psoP", bufs=1, space="PSUM"))

        dsb = pers.tile([HD + 1, S1], f32)           # denom row staging
        dcol = pers.tile([P, P], f32)                # gathered denominators
        drec = pers.tile([P, P], f32)
        drecb = pers.tile([P, P], bf16)

        # Per (head, 1024-col window): a ~12us PE sprint (scores -> exp -> AV,
        # av trailing one m-step), then the denominator/normalize chain on
        # which the next window's PE work waits ~3us -- the HAM drain gap
        # that keeps the sprints at full clock.
        for h in range(HPC):
            c, h2 = h // 2, h % 2
            d0 = h2 * 64
            for np_ in range(NP):
                wsl = slice(np_ * 1024, (np_ + 1) * 1024)
                pso = psoP.tile([HD + 1, 1024], f32, name=f"pso{h}_{np_}",
                                tag="pso")
                ets = {}
                for m in range(M2 + 1):
                    if m < M2:
                        pscs = []
                        for j in range(2):
                            n = np_ * 2 + j
                            psc = ring.tile([P, 512], f32,
                                            name=f"psc{h}_{np_}_{m}_{j}",
                                            tag="ring")
                            nc.tensor.matmul(
                                psc[0:P, :],
                                kT_sb[d0:d0 + 64, c, m * P:(m + 1) * P],
                                qT_sb[d0:d0 + 64, c, n * 512:(n + 1) * 512],
                                start=True, stop=True)
                            pscs.append(psc)
                        for j in range(2):
                            et = expp.tile([P, 512], bf16,
                                           name=f"et{h}_{np_}_{m}_{j}",
                                           tag="expp", bufs=5)
                            nc.scalar.activation(out=et[:], in_=pscs[j][:],
                                                 func=AF.Exp, scale=SCALE)
                            ets[(m, j)] = et
                    if m > 0:
                        mm = m - 1
                        for j in range(2):
                            nc.tensor.matmul(
                                pso[:, j * 512:(j + 1) * 512],
                                v_sb[:, mm, h, :],
                                ets.pop((mm, j))[:],
                                start=(mm == 0), stop=(mm == M2 - 1))
                # ---- denominator -> reciprocal -> broadcast -> normalize ----
                hr = slice(h * 32, h * 32 + 8)   # 32-aligned for DVE ops
                nc.scalar.copy(dsb[HD:HD + 1, wsl], pso[HD:HD + 1, :])
                nc.gpsimd.dma_start(out=dcol[hr, :], in_=dsb[HD:HD + 1, wsl])
                nc.vector.reciprocal(drec[hr, :], dcol[hr, :])
                nc.vector.tensor_copy(drecb[hr, :], drec[hr, :])
                rt1 = rop.tile([1, 1024], bf16, name=f"rcp{h}_{np_}",
                               tag=f"rcp{h}", bufs=1)
                nc.gpsimd.dma_start(out=rt1[:], in_=drecb[hr, :])
                # broadcast recip across 64 partitions: K=1 ones-matmul on PE,
                # staged to SBUF by DVE (the normalize mul may read only one
                # PSUM operand, and that is the AV accumulator)
                prs = rop.tile([HD, 1024], bf16, name=f"prs{h}_{np_}",
                               tag="prs", bufs=2)
                onmt = None
                if h2 == 1:
                    onmt = rop.tile([HD, 1024], bf16, name=f"onm{h}_{np_}",
                                    tag="onm", bufs=2)
                for j in range(2):
                    jsl = slice(np_ * 1024 + j * 512, np_ * 1024 + (j + 1) * 512)
                    lsl = slice(j * 512, (j + 1) * 512)
                    prb = ring.tile([P, 512], f32, name=f"prb{h}_{np_}_{j}",
                                    tag="ring")
                    nc.tensor.matmul(prb[0:HD, :], ones64[:],
                                     rt1[:, lsl], start=True, stop=True)
                    nc.vector.tensor_copy(prs[:, lsl], prb[0:HD, :])
                    if h2 == 0:
                        nc.vector.tensor_mul(onorm[0:HD, c, jsl],
                                             pso[0:HD, lsl], prs[:, lsl])
                    else:
                        nc.vector.tensor_mul(onmt[:, lsl],
                                             pso[0:HD, lsl], prs[:, lsl])
                if h2 == 1:
                    nc.gpsimd.dma_start(out=onorm[HD:P, c, wsl], in_=onmt[:])

        # ---- out-projection tail (PE + ACT drains + sync DMA out) ----
        for np_ in range(NP):
            for mo in range(KC):
                for j in range(2):
                    pout = ring.tile([P, 512], f32,
                                     name=f"po{np_}_{mo}_{j}", tag="ring")
                    jsl = slice(np_ * 1024 + j * 512,
                                np_ * 1024 + (j + 1) * 512)
                    for c in range(MC):
                        nc.tensor.matmul(
                            pout[:], owt_sb[:, c, mo * P:(mo + 1) * P],
                            onorm[:, c, jsl],
                            start=(c == 0), stop=(c == MC - 1))
                    ost = xs.tile([P, 512], bf16, name=f"ost{np_}_{mo}_{j}",
                                  tag="ost")
                    nc.scalar.copy(ost[:], pout[:])
                    nc.sync.dma_start(out=outT[mo * P:(mo + 1) * P, jsl],
                                      in_=ost[:])

    _legalize_waits(nc, mybir, limit=1)
    return nc


def get_nc():
    if "nc" not in _NC_CACHE:
        _NC_CACHE["nc"] = _build_nc()
    return _NC_CACHE["nc"]


def make_in_maps(x, y, q_w, q_b, kv_w, kv_b, qn_w, qn_b, kn_w, kn_b, out_w, out_b):
    import ml_dtypes
    bf = ml_dtypes.bfloat16
    perm = np.concatenate([np.arange(0, HD, 2), np.arange(1, HD, 2)])
    inv_freq = (1.0 / (10000.0 ** (np.arange(0, HD, 2, dtype=np.float32)
                                   / np.float32(HD)))).astype(np.float32)
    ang = np.arange(S1, dtype=np.float32)[None, :] * inv_freq[:, None]
    cos = np.cos(ang).astype(np.float32)           # (32, S1)
    sin = np.sin(ang).astype(np.float32)
    cosf = np.tile(cos, (4, 1)).astype(bf)
    sinf = np.concatenate([-sin, sin, -sin, sin]).astype(bf)
    sel = np.zeros((CPC, HPC), np.float32)
    for h in range(HPC):
        sel[h * HD:(h + 1) * HD, h] = 1.0
    rsel = np.ascontiguousarray(sel.T)
    rselq = (rsel * np.tile(qn_w[perm], HPC)[None, :]).astype(bf)
    rselk = (rsel * np.tile(kn_w[perm], HPC)[None, :]).astype(bf)
    sel = sel.astype(bf)

    in_maps = []
    for core in range(8):
        b, g = divmod(core, 4)
        heads = [HPC * g + i for i in range(HPC)]
        qrows = np.concatenate([h * HD + perm for h in heads])
        vrows = np.concatenate([CIN + h * HD + np.arange(HD) for h in heads])
        ocols = np.concatenate([h * HD + np.arange(HD) for h in heads])
        in_maps.append({
            "xT": np.ascontiguousarray(x[b].T).astype(bf),
            "yT": np.ascontiguousarray(y[b].T).astype(bf),
            "qwt": np.ascontiguousarray(q_w[qrows].T).astype(bf),
            "kwt": np.ascontiguousarray(kv_w[qrows].T).astype(bf),
            "vwt": np.ascontiguousarray(kv_w[vrows].T).astype(bf),
            "owt": np.ascontiguousarray(out_w[:, ocols].T).astype(bf),
            "qb": np.ascontiguousarray(q_b[qrows]),
            "kb": np.ascontiguousarray(kv_b[qrows]),
            "vb": np.ascontiguousarray(kv_b[vrows]),
            "qnb": np.ascontiguousarray(np.tile(qn_b[perm], HPC)),
            "knb": np.ascontiguousarray(np.tile(kn_b[perm], HPC)),
            "cosf": cosf, "sinf": sinf, "sel": sel,
            "rselq": rselq, "rselk": rselk,
        })
    return in_maps


def assemble(parts, out_b):
    result = np.empty((B, S1, CIN), np.float32)
    for b in range(B):
        acc = parts[b * 4].astype(np.float32)
        for g in range(1, 4):
            acc = acc + parts[b * 4 + g].astype(np.float32)
        result[b] = acc.T + out_b[None, :].astype(np.float32)
    return result


def kernel(**inputs):
    args = {k: np.asarray(inputs[k], np.float32) for k in
            ("x", "y", "q_w", "q_b", "kv_w", "kv_b", "qn_w", "qn_b",
             "kn_w", "kn_b", "out_w", "out_b")}
    in_maps = make_in_maps(
        args["x"], args["y"], args["q_w"], args["q_b"], args["kv_w"],
        args["kv_b"], args["qn_w"], args["qn_b"], args["kn_w"], args["kn_b"],
        args["out_w"], args["out_b"])
    from concourse.bass_utils import run_bass_kernel_spmd
    nc = get_nc()
    res = run_bass_kernel_spmd(nc, in_maps, core_ids=list(range(8)))
    parts = [r["outT"] for r in res.results]
    return assemble(parts, args["out_b"])
